# revision 21
# baseline (speedup 1.0000x reference)
"""Graphwise KL loss (segment_reduce) on 8 trn2 NeuronCores.

Strategy:
  Device (O(N) memory-bound work, data-parallel over 8 cores; each core
  streams a contiguous 1/8 slice, inputs host-packed to bf16):
    pr = y_true * weight                      (DVE bf16 2x)
    d  = ln(pr + 1e-37) - ln(y_pred + 1e-8)   (ACT Ln x2 -> bf16, DVE sub)
    e1 = pr * d                               (DVE bf16 2x)
    32-element block sums of e1 and pr        (PE matmul, block-diag ones)
  Host (O(num_graphs) metadata assembly, fp64): reconstruct per-segment
  sums A_g (e1) and B_g (pr) from device block sums + fp64 partial sums
  at segment boundaries; with S_g = max(B_g, EPS):
      total = mean_g (A_g - B_g * ln(S_g)) / S_g

  Inputs are packed into ONE interleaved bf16 DRAM tensor per core laid
  out [tile, partition, {yp,yt,w}, col] with element i = 128*col + part
  inside a tile, so each 32-element block sits in one partition quad and
  PE reduces blocks via matmul.  The stationary is a [128,16] pair of
  block-diagonal ones matrices (cols 0:8 -> psum rows 0:4 for the first
  half of a chunk's columns, cols 8:16 -> rows 4:8 for the second half)
  so a whole chunk accumulates into one [8, w] PSUM region: e1 sums in
  cols 0:w/2, pr sums in w/2:w.  PSUM is evacuated split: ACT copies the
  e1 half, DVE the pr half, both skewed one chunk late so they never
  block the next chunk's front end.  One load DMA + one store DMA per
  chunk; 3-deep input buffers keep the 16 SDMA engines streaming.

  The work is cut into chunks: 7 full tiles of 1024 columns plus 4
  quarter tiles of 256 at the end, so the serial drain chain after the
  last load is short.

  Raw Bass (no Tile): every op carries at most ONE inline sync wait
  (walrus cap); extra deps use standalone wait_ge instructions.
    POOL: load DMAs    DVE: pr/d/e1 + pr-psum evac
    ACT:  Ln x2 + e1-psum evac    PE: 4 matmuls/chunk    SP: store DMAs
"""

import numpy as np

N_TOTAL = 8388608
N_CORES = 8
N_LOCAL = N_TOTAL // N_CORES      # 1048576
P = 128
TILE_F = 1024                     # columns per full tile
N_TILES = N_LOCAL // (P * TILE_F)  # 8
BLK = 32
JBLK = P // BLK                   # 4 blocks per column
N_BLOCKS_LOCAL = N_LOCAL // BLK   # 32768
EPS = 1e-8
TINY = 1e-37

# chunk list: (tile, col0, width) — last tile split into quarters
CHUNKS = [(0, 0, TILE_F // 2), (0, TILE_F // 2, TILE_F // 2)]
CHUNKS += [(t, 0, TILE_F) for t in range(1, N_TILES - 1)]
CHUNKS += [(N_TILES - 1, c0, TILE_F // 4)
           for c0 in range(0, TILE_F, TILE_F // 4)]
NC_CH = len(CHUNKS)               # 12
PRC = TILE_F // 2                 # psum col of the pr group (bank-aligned)

_CACHE = {}


def _check_one_wait(nc):
    """Assert no non-EventSemaphore instruction carries more than one wait."""
    bad = []
    for f in nc.m.functions:
        for bb in f.blocks:
            for inst in bb.instructions:
                si = inst.sync_info
                if si and si.on_wait and len(si.on_wait) > 1:
                    if "EventSem" not in type(inst).__name__:
                        bad.append((type(inst).__name__, inst.name, len(si.on_wait)))
    assert not bad, f"multi-wait instructions remain: {bad}"


def _build_program():
    import concourse.bass as bass
    import concourse.mybir as mybir

    f32 = mybir.dt.float32
    bf16 = mybir.dt.bfloat16
    Ln = mybir.ActivationFunctionType.Ln
    Copy = mybir.ActivationFunctionType.Copy

    nc = bass.Bass()

    # Const APs for the Ln biases (activation() looks these up by value).
    # Emitted on the otherwise-idle DVE so POOL can start load DMAs at once.
    s_cst = nc.alloc_semaphore("s_cst")
    for val in (TINY, EPS):
        ct = nc.alloc_sbuf_tensor(f"const-f32-{val}", [128, 1], f32)
        nc.vector.memset(ct.ap(), val).then_inc(s_cst, 1)
        nc.const_aps.aps[(f32, val)] = ct.ap()

    pk = nc.declare_dram_parameter("pk", [3 * N_LOCAL], bf16, isOutput=False)
    wm = nc.declare_dram_parameter("wm", [P * 16], f32, isOutput=False)
    od = nc.declare_dram_parameter("od", [NC_CH * 8 * TILE_F], f32,
                                   isOutput=True)

    pk3 = pk[:].rearrange("(t p cf) -> t p cf", p=P, cf=3 * TILE_F)
    pk4 = pk[:].rearrange("(t p c f) -> t p c f", p=P, c=3, f=TILE_F)
    wm2 = wm[:].rearrange("(p j) -> p j", j=16)
    od3 = od[:].rearrange("(i r f) -> i r f", r=8, f=TILE_F)

    N_SL = 4
    t_in = [nc.alloc_sbuf_tensor(f"t_in{i}", [P, 3 * TILE_F], bf16).ap()
            for i in range(N_SL)]
    t_in3 = [a.rearrange("p (c f) -> p c f", c=3) for a in t_in]
    t_pr = [nc.alloc_sbuf_tensor(f"t_pr{i}", [P, TILE_F], bf16).ap()
            for i in range(3)]
    t_lp = [nc.alloc_sbuf_tensor(f"t_lp{i}", [P, TILE_F], bf16).ap()
            for i in range(2)]
    t_lq = [nc.alloc_sbuf_tensor(f"t_lq{i}", [P, TILE_F], bf16).ap()
            for i in range(2)]
    t_d = [nc.alloc_sbuf_tensor(f"t_d{i}", [P, TILE_F], bf16).ap()
           for i in range(2)]
    t_e1 = [nc.alloc_sbuf_tensor(f"t_e1{i}", [P, TILE_F], bf16).ap()
            for i in range(2)]
    t_sb = [nc.alloc_sbuf_tensor(f"t_sb{i}", [8, TILE_F], f32).ap()
            for i in range(2)]
    w32 = nc.alloc_sbuf_tensor("w32", [P, 16], f32).ap()
    w16 = nc.alloc_sbuf_tensor("w16", [P, 16], bf16).ap()

    ps = [nc.alloc_psum_tensor(f"ps{i}", [8, TILE_F], f32).ap()
          for i in range(2)]

    s_in = [nc.alloc_semaphore(f"s_in{i}") for i in range(N_SL)]  # +16/load
    s_o = [nc.alloc_semaphore(f"s_o{i}") for i in range(2)]     # +16 per store
    s_wld = nc.alloc_semaphore("s_wld")                         # +16 W load
    s_dve = nc.alloc_semaphore("s_dve")  # +1 per DVE op
    s_act = nc.alloc_semaphore("s_act")  # +1 per ACT op
    s_pe = nc.alloc_semaphore("s_pe")    # +1 per chunk (4th matmul)

    # Op indices along each engine's in-order stream.
    dve_i = {}
    n = 1                                 # w16 convert = 1
    for c in range(NC_CH):
        for o in ("pr", "d", "e1"):
            n += 1
            dve_i[(o, c)] = n
        if c >= 1:
            n += 1
            dve_i[("cpb", c - 1)] = n
    n += 1
    dve_i[("cpb", NC_CH - 1)] = n

    # ACT order: lq runs one chunk ahead so lp's wait on pr(c) never blocks
    # the next chunk's front end; cpa trails one chunk behind.
    act_order = [("lq", 0)]
    for c in range(NC_CH):
        act_order.append(("lp", c))
        if c >= 1:
            act_order.append(("cpa", c - 1))
        if c + 1 < NC_CH:
            act_order.append(("lq", c + 1))
    act_order.append(("cpa", NC_CH - 1))
    act_i = {}
    n = 0
    for o in act_order:
        n += 1
        act_i[o] = n

    def in_wait(c):
        return (s_in[c % N_SL], 16 * (c // N_SL + 1))

    def o_wait(c):
        # store of chunk c retired
        return (s_o[c % 2], 16 * (c // 2 + 1))

    with nc.Block() as block:

        @block.gpsimd
        def _(g):
            # POOL starts behind the Bass-init memsets, so it gets the
            # non-latency-critical work: the W load and the result stores.
            g.dma_start(w32, wm2).then_inc(s_wld, 16)
            for c, (t, c0, w) in enumerate(CHUNKS):
                s2 = c % 2
                h = w // 2
                src = t_sb[s2].rearrange("r (g f) -> r g f", g=2)[:, :, 0:h]
                dst = od3[c, :, 0:w].rearrange("r (g f) -> r g f", g=2)
                g.wait_ge(s_dve, dve_i[("cpb", c)])
                g.dma_start(dst, src) \
                    ._wait_ge(s_act, act_i[("cpa", c)]).then_inc(s_o[s2], 16)
            g.wait_ge(s_o[0], 16 * ((NC_CH + 1) // 2))
            g.wait_ge(s_o[1], 16 * (NC_CH // 2))

        @block.vector
        def _(v):
            v.tensor_copy(w16, w32)._wait_ge(s_wld, 16).then_inc(s_dve, 1)

            def emit_cpb(cc):
                _, _, w = CHUNKS[cc]
                if cc >= 2:
                    v.wait_ge(*o_wait(cc - 2))
                v.tensor_copy(t_sb[cc % 2][:, PRC:PRC + w // 2],
                              ps[cc % 2][:, PRC:PRC + w // 2]) \
                    ._wait_ge(s_pe, cc + 1).then_inc(s_dve, 1)

            for c, (t, c0, w) in enumerate(CHUNKS):
                s3, s2 = c % 3, c % 2
                v.tensor_mul(t_pr[s3][:, 0:w], t_in3[c % N_SL][:, 1, 0:w],
                             t_in3[c % N_SL][:, 2, 0:w]) \
                    ._wait_ge(*in_wait(c)).then_inc(s_dve, 1)
                v.tensor_sub(t_d[s2][:, 0:w], t_lp[s2][:, 0:w],
                             t_lq[s2][:, 0:w]) \
                    ._wait_ge(s_act, act_i[("lp", c)]).then_inc(s_dve, 1)
                # same-engine RAW on d(c) needs an explicit retire wait
                v.wait_ge(s_dve, dve_i[("d", c)])
                ins = v.tensor_mul(t_e1[s2][:, 0:w], t_pr[s3][:, 0:w],
                                   t_d[s2][:, 0:w])
                if c >= 2:
                    ins._wait_ge(s_pe, c - 1)   # mm(c-2) done: slots free
                ins.then_inc(s_dve, 1)
                if c >= 1:
                    emit_cpb(c - 1)
            emit_cpb(NC_CH - 1)

        @block.scalar
        def _(s):
            s.wait_ge(s_cst, 2)

            def emit_cpa(cc):
                _, _, w = CHUNKS[cc]
                if cc >= 2:
                    s.wait_ge(*o_wait(cc - 2))
                s.activation(t_sb[cc % 2][:, 0:w // 2],
                             ps[cc % 2][:, 0:w // 2], Copy) \
                    ._wait_ge(s_pe, cc + 1).then_inc(s_act, 1)

            def emit_lq(cc):
                _, _, w = CHUNKS[cc]
                s.activation(t_lq[cc % 2][:, 0:w],
                             t_in3[cc % N_SL][:, 0, 0:w], Ln, bias=EPS) \
                    ._wait_ge(*in_wait(cc)).then_inc(s_act, 1)

            emit_lq(0)
            for c, (t, c0, w) in enumerate(CHUNKS):
                s3, s2 = c % 3, c % 2
                s.activation(t_lp[s2][:, 0:w], t_pr[s3][:, 0:w], Ln,
                             bias=TINY) \
                    ._wait_ge(s_dve, dve_i[("pr", c)]).then_inc(s_act, 1)
                if c >= 1:
                    emit_cpa(c - 1)
                if c + 1 < NC_CH:
                    emit_lq(c + 1)
            emit_cpa(NC_CH - 1)

        @block.tensor
        def _(p):
            w8a = w16[:, 0:8]
            w8b = w16[:, 8:16]
            # ldweights of the first matmul precedes its inline wait; order
            # it after the w16 conversion explicitly
            p.wait_ge(s_dve, 1)
            for c, (t, c0, w) in enumerate(CHUNKS):
                s3, s2 = c % 3, c % 2
                h = w // 2
                if c >= 2:
                    # psum slot free once cpa(c-2) retired (cpb via s_dve)
                    p.wait_ge(s_act, act_i[("cpa", c - 2)])
                p.matmul(ps[s2][:, 0:h], w8a, t_e1[s2][:, 0:h],
                         start=True, stop=False) \
                    ._wait_ge(s_dve, dve_i[("e1", c)])
                p.matmul(ps[s2][:, 0:h], w8b, t_e1[s2][:, h:w],
                         start=False, stop=True)
                p.matmul(ps[s2][:, PRC:PRC + h], w8a, t_pr[s3][:, 0:h],
                         start=True, stop=False)
                p.matmul(ps[s2][:, PRC:PRC + h], w8b, t_pr[s3][:, h:w],
                         start=False, stop=True).then_inc(s_pe, 1)

        @block.sync
        def _(sp):
            # SP's stream is otherwise empty, so its first instruction runs
            # right after the init barrier: issue the input loads here.
            for c, (t, c0, w) in enumerate(CHUNKS):
                if w == TILE_F:
                    # flat 2D AP: one contiguous 6KB row per partition
                    ins = sp.dma_start(t_in[c % N_SL], pk3[t, :, :])
                else:
                    ins = sp.dma_start(t_in3[c % N_SL][:, :, 0:w],
                                       pk4[t, :, :, c0:c0 + w])
                if c >= N_SL:
                    # input slot free once lp(c-N_SL) done (covers lq + pr)
                    ins._wait_ge(s_act, act_i[("lp", c - N_SL)])
                ins.then_inc(s_in[c % N_SL], 16)

    _check_one_wait(nc)
    return nc


def _get_program():
    if "nc" not in _CACHE:
        _CACHE["nc"] = _build_program()
    return _CACHE["nc"]


def _pack_inputs(yp, yt, w):
    """[N_TOTAL] f32 x3 -> per-core packed bf16 [t, p, {yp,yt,w}, f]."""
    import ml_dtypes

    def to_tiles(x):
        # element i_local = t*P*TILE_F + f*P + p  ->  [core, t, p, f]
        return x.reshape(N_CORES, N_TILES, TILE_F, P).transpose(0, 1, 3, 2)

    pk = np.stack([to_tiles(yp), to_tiles(yt), to_tiles(w)], axis=3)
    pk = np.ascontiguousarray(pk).astype(ml_dtypes.bfloat16)
    return pk.reshape(N_CORES, -1)


_WMAT = None


def _wmat():
    global _WMAT
    if _WMAT is None:
        wmat = np.zeros((P, 16), dtype=np.float32)
        for j in range(JBLK):
            wmat[BLK * j:BLK * (j + 1), j] = 1.0        # half 0 -> rows 0:4
            wmat[BLK * j:BLK * (j + 1), 12 + j] = 1.0   # half 1 -> rows 4:8
        _WMAT = wmat.reshape(-1)
    return _WMAT


def _run_device(yp, yt, w, trace=False):
    from concourse.bass_utils import run_bass_kernel_spmd

    nc = _get_program()
    pk = _pack_inputs(yp, yt, w)
    wmat = _wmat()
    in_maps = [{"pk": pk[k], "wm": wmat} for k in range(N_CORES)]
    res = run_bass_kernel_spmd(nc, in_maps, list(range(N_CORES)), trace=trace)

    bs1_parts, bs2_parts = [], []
    for r in res.results:
        dev = r["od"].reshape(NC_CH, 8, TILE_F)
        bs1 = np.empty(N_BLOCKS_LOCAL, dtype=np.float64)
        bs2 = np.empty(N_BLOCKS_LOCAL, dtype=np.float64)
        for c, (t, c0, w) in enumerate(CHUNKS):
            h = w // 2
            # psum row r = quad j + 4*half; block = t*4096 + (c0+h*half+f')*4+j
            base = t * (TILE_F * JBLK) + c0 * JBLK
            nblk = w * JBLK
            b1 = dev[c, :, 0:h].reshape(2, JBLK, h)
            b2 = dev[c, :, h:w].reshape(2, JBLK, h)
            bs1[base:base + nblk] = b1.transpose(0, 2, 1).reshape(-1)
            bs2[base:base + nblk] = b2.transpose(0, 2, 1).reshape(-1)
        bs1_parts.append(bs1)
        bs2_parts.append(bs2)
    return np.concatenate(bs1_parts), np.concatenate(bs2_parts), res


def kernel(y_pred, y_true, weight, segment_ptr, _trace=False):
    yp = np.ascontiguousarray(np.asarray(y_pred), dtype=np.float32).reshape(-1)
    yt = np.ascontiguousarray(np.asarray(y_true), dtype=np.float32).reshape(-1)
    w = np.ascontiguousarray(np.asarray(weight), dtype=np.float32).reshape(-1)
    ptr = np.asarray(segment_ptr).astype(np.int64).reshape(-1)
    n = yp.shape[0]
    G = ptr.shape[0] - 1
    assert n == N_TOTAL, f"kernel compiled for N={N_TOTAL}, got {n}"

    bs1, bs2, res = _run_device(yp, yt, w, trace=_trace)
    _CACHE["last_res"] = res

    # ---- host assembly in fp64 ----
    pre1 = np.empty(bs1.shape[0] + 1)
    pre1[0] = 0.0
    np.cumsum(bs1, dtype=np.float64, out=pre1[1:])
    pre2 = np.empty(bs2.shape[0] + 1)
    pre2[0] = 0.0
    np.cumsum(bs2, dtype=np.float64, out=pre2[1:])

    # clip ptr defensively to [0, n] (reference guarantees this range)
    ptrc = np.clip(ptr, 0, n)
    b_idx = ptrc // BLK
    r = ptrc - b_idx * BLK  # offset within block
    # fp64 partial sums over [ptr - r, ptr) for boundaries not block-aligned
    seg_off = np.concatenate([[0], np.cumsum(r)])
    tot = int(seg_off[-1])
    part1 = np.zeros(ptrc.shape[0])
    part2 = np.zeros(ptrc.shape[0])
    if tot > 0:
        idx = np.repeat(ptrc - r, r) + (np.arange(tot) - np.repeat(seg_off[:-1], r))
        pr_h = yt[idx].astype(np.float64) * w[idx].astype(np.float64)
        e1_h = pr_h * (np.log(pr_h + TINY) - np.log(yp[idx].astype(np.float64) + EPS))
        nz = r > 0
        red_idx = np.minimum(seg_off[:-1][nz], tot - 1).astype(np.int64)
        part1[nz] = np.add.reduceat(e1_h, red_idx)
        part2[nz] = np.add.reduceat(pr_h, red_idx)

    C1 = pre1[b_idx] + part1
    C2 = pre2[b_idx] + part2
    A = np.diff(C1)
    Bg = np.diff(C2)
    S = np.maximum(Bg, EPS)
    total = np.sum((A - Bg * np.log(S)) / S) / max(G, 1)
    return np.float32(total)


# revision 24
# speedup vs baseline: 1.0121x; 1.0121x over previous
"""Graphwise KL loss (segment_reduce) on 8 trn2 NeuronCores.

Strategy:
  Device (O(N) memory-bound work, data-parallel over 8 cores; each core
  streams a contiguous 1/8 slice, inputs host-packed to bf16):
    pr = y_true * weight                      (DVE bf16 2x)
    d  = ln(pr + 1e-37) - ln(y_pred + 1e-8)   (ACT Ln x2 -> bf16, DVE sub)
    e1 = pr * d                               (DVE bf16 2x)
    32-element block sums of e1 and pr        (PE matmul, block-diag ones)
  Host (O(num_graphs) metadata assembly, fp64): reconstruct per-segment
  sums A_g (e1) and B_g (pr) from device block sums + fp64 partial sums
  at segment boundaries; with S_g = max(B_g, EPS):
      total = mean_g (A_g - B_g * ln(S_g)) / S_g

  Inputs are packed into ONE interleaved bf16 DRAM tensor per core laid
  out [tile, partition, {yp,yt,w}, col] with element i = 128*col + part
  inside a tile, so each 32-element block sits in one partition quad and
  PE reduces blocks via matmul.  The stationary is a [128,16] pair of
  block-diagonal ones matrices (cols 0:8 -> psum rows 0:4 for the first
  half of a chunk's columns, cols 8:16 -> rows 4:8 for the second half)
  so a whole chunk accumulates into one [8, w] PSUM region: e1 sums in
  cols 0:w/2, pr sums in w/2:w.  PSUM is evacuated split: ACT copies the
  e1 half, DVE the pr half, both skewed one chunk late so they never
  block the next chunk's front end.  One load DMA + one store DMA per
  chunk; 3-deep input buffers keep the 16 SDMA engines streaming.

  The work is cut into chunks: 7 full tiles of 1024 columns plus 4
  quarter tiles of 256 at the end, so the serial drain chain after the
  last load is short.

  Raw Bass (no Tile): every op carries at most ONE inline sync wait
  (walrus cap); extra deps use standalone wait_ge instructions.
    POOL: load DMAs    DVE: pr/d/e1 + pr-psum evac
    ACT:  Ln x2 + e1-psum evac    PE: 4 matmuls/chunk    SP: store DMAs
"""

import numpy as np

N_TOTAL = 8388608
N_CORES = 8
N_LOCAL = N_TOTAL // N_CORES      # 1048576
P = 128
TILE_F = 1024                     # columns per full tile
N_TILES = N_LOCAL // (P * TILE_F)  # 8
BLK = 32
JBLK = P // BLK                   # 4 blocks per column
N_BLOCKS_LOCAL = N_LOCAL // BLK   # 32768
EPS = 1e-8
TINY = 1e-37

# chunk list: (tile, col0, width) — last tile split into quarters
CHUNKS = [(0, 0, TILE_F // 2), (0, TILE_F // 2, TILE_F // 2)]
CHUNKS += [(t, 0, TILE_F) for t in range(1, N_TILES - 1)]
CHUNKS += [(N_TILES - 1, c0, TILE_F // 4)
           for c0 in range(0, TILE_F, TILE_F // 4)]
NC_CH = len(CHUNKS)               # 12
PRC = TILE_F // 2                 # psum col of the pr group (bank-aligned)

_CACHE = {}


def _check_one_wait(nc):
    """Assert no non-EventSemaphore instruction carries more than one wait."""
    bad = []
    for f in nc.m.functions:
        for bb in f.blocks:
            for inst in bb.instructions:
                si = inst.sync_info
                if si and si.on_wait and len(si.on_wait) > 1:
                    if "EventSem" not in type(inst).__name__:
                        bad.append((type(inst).__name__, inst.name, len(si.on_wait)))
    assert not bad, f"multi-wait instructions remain: {bad}"


def _build_program():
    import concourse.bass as bass
    import concourse.mybir as mybir

    f32 = mybir.dt.float32
    bf16 = mybir.dt.bfloat16
    Ln = mybir.ActivationFunctionType.Ln
    Copy = mybir.ActivationFunctionType.Copy

    nc = bass.Bass()

    # Const APs for the Ln biases (activation() looks these up by value).
    # Emitted on the otherwise-idle DVE so POOL can start load DMAs at once.
    s_cst = nc.alloc_semaphore("s_cst")
    for val in (TINY, EPS):
        ct = nc.alloc_sbuf_tensor(f"const-f32-{val}", [128, 1], f32)
        nc.vector.memset(ct.ap(), val).then_inc(s_cst, 1)
        nc.const_aps.aps[(f32, val)] = ct.ap()

    pk = nc.declare_dram_parameter("pk", [3 * N_LOCAL], bf16, isOutput=False)
    wm = nc.declare_dram_parameter("wm", [P * 16], f32, isOutput=False)
    od = nc.declare_dram_parameter("od", [NC_CH * 8 * TILE_F], f32,
                                   isOutput=True)

    pk3 = pk[:].rearrange("(t p cf) -> t p cf", p=P, cf=3 * TILE_F)
    pk4 = pk[:].rearrange("(t p c f) -> t p c f", p=P, c=3, f=TILE_F)
    wm2 = wm[:].rearrange("(p j) -> p j", j=16)
    od3 = od[:].rearrange("(i r f) -> i r f", r=8, f=TILE_F)

    N_SL = 4
    t_in = [nc.alloc_sbuf_tensor(f"t_in{i}", [P, 3 * TILE_F], bf16).ap()
            for i in range(N_SL)]
    t_in3 = [a.rearrange("p (c f) -> p c f", c=3) for a in t_in]
    t_pr = [nc.alloc_sbuf_tensor(f"t_pr{i}", [P, TILE_F], bf16).ap()
            for i in range(3)]
    t_lp = [nc.alloc_sbuf_tensor(f"t_lp{i}", [P, TILE_F], bf16).ap()
            for i in range(2)]
    t_lq = [nc.alloc_sbuf_tensor(f"t_lq{i}", [P, TILE_F], bf16).ap()
            for i in range(2)]
    t_d = [nc.alloc_sbuf_tensor(f"t_d{i}", [P, TILE_F], bf16).ap()
           for i in range(2)]
    t_e1 = [nc.alloc_sbuf_tensor(f"t_e1{i}", [P, TILE_F], bf16).ap()
            for i in range(2)]
    t_sb = [nc.alloc_sbuf_tensor(f"t_sb{i}", [8, TILE_F], f32).ap()
            for i in range(2)]
    w32 = nc.alloc_sbuf_tensor("w32", [P, 16], f32).ap()
    w16 = nc.alloc_sbuf_tensor("w16", [P, 16], bf16).ap()

    ps = [nc.alloc_psum_tensor(f"ps{i}", [8, TILE_F], f32).ap()
          for i in range(2)]

    s_in = [nc.alloc_semaphore(f"s_in{i}") for i in range(N_SL)]  # +16/load
    s_o = [nc.alloc_semaphore(f"s_o{i}") for i in range(2)]     # +16 per store
    s_wld = nc.alloc_semaphore("s_wld")                         # +16 W load
    s_dve = nc.alloc_semaphore("s_dve")  # +1 per DVE op
    s_act = nc.alloc_semaphore("s_act")  # +1 per ACT op
    s_pe = nc.alloc_semaphore("s_pe")    # +1 per chunk (4th matmul)

    # Op indices along each engine's in-order stream.
    # DVE order hoists pr(c+1) right after d(c) so ACT's lp(c+1) can start
    # while DVE still runs e1(c)/cpb(c-1) — this breaks the cross-engine
    # latency loop that otherwise sets the cycle time.
    dve_order = [("pr", 0)]
    for c in range(NC_CH):
        dve_order.append(("d", c))
        if c + 1 < NC_CH:
            dve_order.append(("pr", c + 1))
        dve_order.append(("e1", c))
        if c >= 1:
            dve_order.append(("cpb", c - 1))
    dve_order.append(("cpb", NC_CH - 1))
    dve_i = {}
    n = 1                                 # w16 convert = 1
    for o in dve_order:
        n += 1
        dve_i[o] = n

    # ACT order: lq runs one chunk ahead so lp's wait on pr(c) never blocks
    # the next chunk's front end; cpa trails one chunk behind.
    act_order = [("lq", 0)]
    for c in range(NC_CH):
        act_order.append(("lp", c))
        if c >= 1:
            act_order.append(("cpa", c - 1))
        if c + 1 < NC_CH:
            act_order.append(("lq", c + 1))
    act_order.append(("cpa", NC_CH - 1))
    act_i = {}
    n = 0
    for o in act_order:
        n += 1
        act_i[o] = n

    def in_wait(c):
        return (s_in[c % N_SL], 16 * (c // N_SL + 1))

    def o_wait(c):
        # store of chunk c retired
        return (s_o[c % 2], 16 * (c // 2 + 1))

    with nc.Block() as block:

        @block.gpsimd
        def _(g):
            # POOL starts behind the Bass-init memsets, so it gets the
            # non-latency-critical work: the W load and the result stores.
            g.dma_start(w32, wm2).then_inc(s_wld, 16)
            for c, (t, c0, w) in enumerate(CHUNKS):
                s2 = c % 2
                h = w // 2
                src = t_sb[s2].rearrange("r (g f) -> r g f", g=2)[:, :, 0:h]
                dst = od3[c, :, 0:w].rearrange("r (g f) -> r g f", g=2)
                g.wait_ge(s_dve, dve_i[("cpb", c)])
                g.dma_start(dst, src) \
                    ._wait_ge(s_act, act_i[("cpa", c)]).then_inc(s_o[s2], 16)
            g.wait_ge(s_o[0], 16 * ((NC_CH + 1) // 2))
            g.wait_ge(s_o[1], 16 * (NC_CH // 2))

        @block.vector
        def _(v):
            v.tensor_copy(w16, w32)._wait_ge(s_wld, 16).then_inc(s_dve, 1)

            def emit_cpb(cc):
                _, _, w = CHUNKS[cc]
                if cc >= 2:
                    v.wait_ge(*o_wait(cc - 2))
                v.tensor_copy(t_sb[cc % 2][:, PRC:PRC + w // 2],
                              ps[cc % 2][:, PRC:PRC + w // 2]) \
                    ._wait_ge(s_pe, cc + 1).then_inc(s_dve, 1)

            def emit_pr(cc):
                _, _, w = CHUNKS[cc]
                if cc >= 3:
                    # pr slot free once mm(cc-3) retired; the same wait
                    # (s_pe >= cc-2) covers e1(cc-1)'s slot WAR
                    v.wait_ge(s_pe, cc - 2)
                v.tensor_mul(t_pr[cc % 3][:, 0:w],
                             t_in3[cc % N_SL][:, 1, 0:w],
                             t_in3[cc % N_SL][:, 2, 0:w]) \
                    ._wait_ge(*in_wait(cc)).then_inc(s_dve, 1)

            emit_pr(0)
            for c, (t, c0, w) in enumerate(CHUNKS):
                s3, s2 = c % 3, c % 2
                v.tensor_sub(t_d[s2][:, 0:w], t_lp[s2][:, 0:w],
                             t_lq[s2][:, 0:w]) \
                    ._wait_ge(s_act, act_i[("lp", c)]).then_inc(s_dve, 1)
                if c + 1 < NC_CH:
                    emit_pr(c + 1)
                else:
                    # no pr lookahead on the last chunk: cover e1's WAR
                    v.wait_ge(s_pe, c - 1)
                # same-engine RAW on d(c) needs an explicit retire wait
                v.wait_ge(s_dve, dve_i[("d", c)])
                v.tensor_mul(t_e1[s2][:, 0:w], t_pr[s3][:, 0:w],
                             t_d[s2][:, 0:w]).then_inc(s_dve, 1)
                if c >= 1:
                    emit_cpb(c - 1)
            emit_cpb(NC_CH - 1)

        @block.scalar
        def _(s):
            s.wait_ge(s_cst, 2)

            def emit_cpa(cc):
                _, _, w = CHUNKS[cc]
                if cc >= 2:
                    s.wait_ge(*o_wait(cc - 2))
                s.activation(t_sb[cc % 2][:, 0:w // 2],
                             ps[cc % 2][:, 0:w // 2], Copy) \
                    ._wait_ge(s_pe, cc + 1).then_inc(s_act, 1)

            def emit_lq(cc):
                _, _, w = CHUNKS[cc]
                s.activation(t_lq[cc % 2][:, 0:w],
                             t_in3[cc % N_SL][:, 0, 0:w], Ln, bias=EPS) \
                    ._wait_ge(*in_wait(cc)).then_inc(s_act, 1)

            emit_lq(0)
            for c, (t, c0, w) in enumerate(CHUNKS):
                s3, s2 = c % 3, c % 2
                s.activation(t_lp[s2][:, 0:w], t_pr[s3][:, 0:w], Ln,
                             bias=TINY) \
                    ._wait_ge(s_dve, dve_i[("pr", c)]).then_inc(s_act, 1)
                if c >= 1:
                    emit_cpa(c - 1)
                if c + 1 < NC_CH:
                    emit_lq(c + 1)
            emit_cpa(NC_CH - 1)

        @block.tensor
        def _(p):
            w8a = w16[:, 0:8]
            w8b = w16[:, 8:16]
            # ldweights of the first matmul precedes its inline wait; order
            # it after the w16 conversion explicitly
            p.wait_ge(s_dve, 1)
            for c, (t, c0, w) in enumerate(CHUNKS):
                s3, s2 = c % 3, c % 2
                h = w // 2
                if c >= 2:
                    # psum slot free once cpa(c-2) retired (cpb via s_dve)
                    p.wait_ge(s_act, act_i[("cpa", c - 2)])
                p.matmul(ps[s2][:, 0:h], w8a, t_e1[s2][:, 0:h],
                         start=True, stop=False) \
                    ._wait_ge(s_dve, dve_i[("e1", c)])
                p.matmul(ps[s2][:, 0:h], w8b, t_e1[s2][:, h:w],
                         start=False, stop=True)
                p.matmul(ps[s2][:, PRC:PRC + h], w8a, t_pr[s3][:, 0:h],
                         start=True, stop=False)
                p.matmul(ps[s2][:, PRC:PRC + h], w8b, t_pr[s3][:, h:w],
                         start=False, stop=True).then_inc(s_pe, 1)

        @block.sync
        def _(sp):
            # SP's stream is otherwise empty, so its first instruction runs
            # right after the init barrier: issue the input loads here.
            for c, (t, c0, w) in enumerate(CHUNKS):
                if w == TILE_F:
                    # flat 2D AP: one contiguous 6KB row per partition
                    ins = sp.dma_start(t_in[c % N_SL], pk3[t, :, :])
                else:
                    ins = sp.dma_start(t_in3[c % N_SL][:, :, 0:w],
                                       pk4[t, :, :, c0:c0 + w])
                if c >= N_SL:
                    # input slot free once lp(c-N_SL) done (covers lq + pr)
                    ins._wait_ge(s_act, act_i[("lp", c - N_SL)])
                ins.then_inc(s_in[c % N_SL], 16)

    _check_one_wait(nc)
    return nc


def _get_program():
    if "nc" not in _CACHE:
        _CACHE["nc"] = _build_program()
    return _CACHE["nc"]


def _pack_inputs(yp, yt, w):
    """[N_TOTAL] f32 x3 -> per-core packed bf16 [t, p, {yp,yt,w}, f]."""
    import ml_dtypes

    def to_tiles(x):
        # element i_local = t*P*TILE_F + f*P + p  ->  [core, t, p, f]
        return x.reshape(N_CORES, N_TILES, TILE_F, P).transpose(0, 1, 3, 2)

    pk = np.stack([to_tiles(yp), to_tiles(yt), to_tiles(w)], axis=3)
    pk = np.ascontiguousarray(pk).astype(ml_dtypes.bfloat16)
    return pk.reshape(N_CORES, -1)


_WMAT = None


def _wmat():
    global _WMAT
    if _WMAT is None:
        wmat = np.zeros((P, 16), dtype=np.float32)
        for j in range(JBLK):
            wmat[BLK * j:BLK * (j + 1), j] = 1.0        # half 0 -> rows 0:4
            wmat[BLK * j:BLK * (j + 1), 12 + j] = 1.0   # half 1 -> rows 4:8
        _WMAT = wmat.reshape(-1)
    return _WMAT


def _run_device(yp, yt, w, trace=False):
    from concourse.bass_utils import run_bass_kernel_spmd

    nc = _get_program()
    pk = _pack_inputs(yp, yt, w)
    wmat = _wmat()
    in_maps = [{"pk": pk[k], "wm": wmat} for k in range(N_CORES)]
    res = run_bass_kernel_spmd(nc, in_maps, list(range(N_CORES)), trace=trace)

    bs1_parts, bs2_parts = [], []
    for r in res.results:
        dev = r["od"].reshape(NC_CH, 8, TILE_F)
        bs1 = np.empty(N_BLOCKS_LOCAL, dtype=np.float64)
        bs2 = np.empty(N_BLOCKS_LOCAL, dtype=np.float64)
        for c, (t, c0, w) in enumerate(CHUNKS):
            h = w // 2
            # psum row r = quad j + 4*half; block = t*4096 + (c0+h*half+f')*4+j
            base = t * (TILE_F * JBLK) + c0 * JBLK
            nblk = w * JBLK
            b1 = dev[c, :, 0:h].reshape(2, JBLK, h)
            b2 = dev[c, :, h:w].reshape(2, JBLK, h)
            bs1[base:base + nblk] = b1.transpose(0, 2, 1).reshape(-1)
            bs2[base:base + nblk] = b2.transpose(0, 2, 1).reshape(-1)
        bs1_parts.append(bs1)
        bs2_parts.append(bs2)
    return np.concatenate(bs1_parts), np.concatenate(bs2_parts), res


def kernel(y_pred, y_true, weight, segment_ptr, _trace=False):
    yp = np.ascontiguousarray(np.asarray(y_pred), dtype=np.float32).reshape(-1)
    yt = np.ascontiguousarray(np.asarray(y_true), dtype=np.float32).reshape(-1)
    w = np.ascontiguousarray(np.asarray(weight), dtype=np.float32).reshape(-1)
    ptr = np.asarray(segment_ptr).astype(np.int64).reshape(-1)
    n = yp.shape[0]
    G = ptr.shape[0] - 1
    assert n == N_TOTAL, f"kernel compiled for N={N_TOTAL}, got {n}"

    bs1, bs2, res = _run_device(yp, yt, w, trace=_trace)
    _CACHE["last_res"] = res

    # ---- host assembly in fp64 ----
    pre1 = np.empty(bs1.shape[0] + 1)
    pre1[0] = 0.0
    np.cumsum(bs1, dtype=np.float64, out=pre1[1:])
    pre2 = np.empty(bs2.shape[0] + 1)
    pre2[0] = 0.0
    np.cumsum(bs2, dtype=np.float64, out=pre2[1:])

    # clip ptr defensively to [0, n] (reference guarantees this range)
    ptrc = np.clip(ptr, 0, n)
    b_idx = ptrc // BLK
    r = ptrc - b_idx * BLK  # offset within block
    # fp64 partial sums over [ptr - r, ptr) for boundaries not block-aligned
    seg_off = np.concatenate([[0], np.cumsum(r)])
    tot = int(seg_off[-1])
    part1 = np.zeros(ptrc.shape[0])
    part2 = np.zeros(ptrc.shape[0])
    if tot > 0:
        idx = np.repeat(ptrc - r, r) + (np.arange(tot) - np.repeat(seg_off[:-1], r))
        pr_h = yt[idx].astype(np.float64) * w[idx].astype(np.float64)
        e1_h = pr_h * (np.log(pr_h + TINY) - np.log(yp[idx].astype(np.float64) + EPS))
        nz = r > 0
        red_idx = np.minimum(seg_off[:-1][nz], tot - 1).astype(np.int64)
        part1[nz] = np.add.reduceat(e1_h, red_idx)
        part2[nz] = np.add.reduceat(pr_h, red_idx)

    C1 = pre1[b_idx] + part1
    C2 = pre2[b_idx] + part2
    A = np.diff(C1)
    Bg = np.diff(C2)
    S = np.maximum(Bg, EPS)
    total = np.sum((A - Bg * np.log(S)) / S) / max(G, 1)
    return np.float32(total)


# revision 25
# speedup vs baseline: 1.1748x; 1.1608x over previous
"""Graphwise KL loss (segment_reduce) on 8 trn2 NeuronCores.

Strategy:
  Device (O(N) memory-bound work, data-parallel over 8 cores; each core
  streams a contiguous 1/8 slice, inputs host-packed to bf16):
    pr = y_true * weight                      (DVE bf16 2x)
    d  = ln(pr + 1e-37) - ln(y_pred + 1e-8)   (ACT Ln x2 -> bf16, DVE sub)
    e1 = pr * d                               (DVE bf16 2x)
    32-element block sums of e1 and pr        (PE matmul, block-diag ones)
  Host (O(num_graphs) metadata assembly, fp64): reconstruct per-segment
  sums A_g (e1) and B_g (pr) from device block sums + fp64 partial sums
  at segment boundaries; with S_g = max(B_g, EPS):
      total = mean_g (A_g - B_g * ln(S_g)) / S_g

  Inputs are packed into ONE interleaved bf16 DRAM tensor per core laid
  out [tile, partition, {yp,yt,w}, col] with element i = 128*col + part
  inside a tile, so each 32-element block sits in one partition quad and
  PE reduces blocks via matmul.  The stationary is a [128,16] pair of
  block-diagonal ones matrices (cols 0:8 -> psum rows 0:4 for the first
  half of a chunk's columns, cols 8:16 -> rows 4:8 for the second half)
  so a whole chunk accumulates into one [8, w] PSUM region: e1 sums in
  cols 0:w/2, pr sums in w/2:w.  PSUM is evacuated split: ACT copies the
  e1 half, DVE the pr half, both skewed one chunk late so they never
  block the next chunk's front end.  One load DMA + one store DMA per
  chunk; 3-deep input buffers keep the 16 SDMA engines streaming.

  The work is cut into chunks: 7 full tiles of 1024 columns plus 4
  quarter tiles of 256 at the end, so the serial drain chain after the
  last load is short.

  Raw Bass (no Tile): every op carries at most ONE inline sync wait
  (walrus cap); extra deps use standalone wait_ge instructions.
    POOL: load DMAs    DVE: pr/d/e1 + pr-psum evac
    ACT:  Ln x2 + e1-psum evac    PE: 4 matmuls/chunk    SP: store DMAs
"""

import numpy as np

N_TOTAL = 8388608
N_CORES = 8
N_LOCAL = N_TOTAL // N_CORES      # 1048576
P = 128
TILE_F = 1024                     # columns per full tile
N_TILES = N_LOCAL // (P * TILE_F)  # 8
BLK = 32
JBLK = P // BLK                   # 4 blocks per column
N_BLOCKS_LOCAL = N_LOCAL // BLK   # 32768
EPS = 1e-8
TINY = 1e-37

# chunk list: (tile, col0, width) — last tile split into quarters
CHUNKS = [(0, 0, TILE_F // 2), (0, TILE_F // 2, TILE_F // 2)]
CHUNKS += [(t, 0, TILE_F) for t in range(1, N_TILES - 1)]
CHUNKS += [(N_TILES - 1, c0, TILE_F // 4)
           for c0 in range(0, TILE_F, TILE_F // 4)]
NC_CH = len(CHUNKS)               # 12
PRC = TILE_F // 2                 # psum col of the pr group (bank-aligned)

_CACHE = {}


def _check_one_wait(nc):
    """Assert no non-EventSemaphore instruction carries more than one wait."""
    bad = []
    for f in nc.m.functions:
        for bb in f.blocks:
            for inst in bb.instructions:
                si = inst.sync_info
                if si and si.on_wait and len(si.on_wait) > 1:
                    if "EventSem" not in type(inst).__name__:
                        bad.append((type(inst).__name__, inst.name, len(si.on_wait)))
    assert not bad, f"multi-wait instructions remain: {bad}"


def _build_program():
    import concourse.bass as bass
    import concourse.mybir as mybir

    f32 = mybir.dt.float32
    bf16 = mybir.dt.bfloat16
    Ln = mybir.ActivationFunctionType.Ln
    Copy = mybir.ActivationFunctionType.Copy

    nc = bass.Bass()

    # Const APs for the Ln biases (activation() looks these up by value).
    # Emitted on the otherwise-idle DVE so POOL can start load DMAs at once.
    s_cst = nc.alloc_semaphore("s_cst")
    for val in (TINY, EPS):
        ct = nc.alloc_sbuf_tensor(f"const-f32-{val}", [128, 1], f32)
        nc.vector.memset(ct.ap(), val).then_inc(s_cst, 1)
        nc.const_aps.aps[(f32, val)] = ct.ap()

    pk = nc.declare_dram_parameter("pk", [3 * N_LOCAL], bf16, isOutput=False)
    wm = nc.declare_dram_parameter("wm", [P * 16], f32, isOutput=False)
    od = nc.declare_dram_parameter("od", [NC_CH * 8 * TILE_F], f32,
                                   isOutput=True)

    pk3 = pk[:].rearrange("(t p cf) -> t p cf", p=P, cf=3 * TILE_F)
    pk4 = pk[:].rearrange("(t p c f) -> t p c f", p=P, c=3, f=TILE_F)
    wm2 = wm[:].rearrange("(p j) -> p j", j=16)
    od3 = od[:].rearrange("(i r f) -> i r f", r=8, f=TILE_F)

    N_SL = 4
    t_in = [nc.alloc_sbuf_tensor(f"t_in{i}", [P, 3 * TILE_F], bf16).ap()
            for i in range(N_SL)]
    t_in3 = [a.rearrange("p (c f) -> p c f", c=3) for a in t_in]
    t_pr = [nc.alloc_sbuf_tensor(f"t_pr{i}", [P, TILE_F], bf16).ap()
            for i in range(3)]
    t_lp = [nc.alloc_sbuf_tensor(f"t_lp{i}", [P, TILE_F], bf16).ap()
            for i in range(2)]
    t_lq = [nc.alloc_sbuf_tensor(f"t_lq{i}", [P, TILE_F], bf16).ap()
            for i in range(2)]
    t_d = [nc.alloc_sbuf_tensor(f"t_d{i}", [P, TILE_F], bf16).ap()
           for i in range(2)]
    t_e1 = [nc.alloc_sbuf_tensor(f"t_e1{i}", [P, TILE_F], bf16).ap()
            for i in range(2)]
    t_sb = [nc.alloc_sbuf_tensor(f"t_sb{i}", [8, TILE_F], f32).ap()
            for i in range(3)]
    w32 = nc.alloc_sbuf_tensor("w32", [P, 16], f32).ap()
    w16 = nc.alloc_sbuf_tensor("w16", [P, 16], bf16).ap()

    ps = [nc.alloc_psum_tensor(f"ps{i}", [8, TILE_F], f32).ap()
          for i in range(3)]

    s_in = [nc.alloc_semaphore(f"s_in{i}") for i in range(N_SL)]  # +16/load
    s_o = [nc.alloc_semaphore(f"s_o{i}") for i in range(2)]     # +16 per store
    s_wld = nc.alloc_semaphore("s_wld")                         # +16 W load
    s_dve = nc.alloc_semaphore("s_dve")  # +1 per DVE op
    s_act = nc.alloc_semaphore("s_act")  # +1 per ACT op
    s_pe = nc.alloc_semaphore("s_pe")    # +1 per chunk (4th matmul)

    # Op indices along each engine's in-order stream.
    # DVE order hoists pr(c+1) right after d(c) so ACT's lp(c+1) can start
    # while DVE still runs e1(c)/cpb(c-1) — this breaks the cross-engine
    # latency loop that otherwise sets the cycle time.
    dve_order = [("pr", 0)]
    for c in range(NC_CH):
        dve_order.append(("d", c))
        if c + 1 < NC_CH:
            dve_order.append(("pr", c + 1))
        dve_order.append(("e1", c))
        if c >= 2:
            dve_order.append(("cpb", c - 2))
    dve_order.append(("cpb", NC_CH - 2))
    dve_order.append(("cpb", NC_CH - 1))
    dve_i = {}
    n = 1                                 # w16 convert = 1
    for o in dve_order:
        n += 1
        dve_i[o] = n

    # ACT order: lq runs one chunk ahead so lp's wait on pr(c) never blocks
    # the next chunk's front end; cpa trails one chunk behind.
    act_order = [("lq", 0)]
    for c in range(NC_CH):
        act_order.append(("lp", c))
        if c >= 2:
            act_order.append(("cpa", c - 2))
        if c + 1 < NC_CH:
            act_order.append(("lq", c + 1))
    act_order.append(("cpa", NC_CH - 2))
    act_order.append(("cpa", NC_CH - 1))
    act_i = {}
    n = 0
    for o in act_order:
        n += 1
        act_i[o] = n

    def in_wait(c):
        return (s_in[c % N_SL], 16 * (c // N_SL + 1))

    def o_wait(c):
        # store of chunk c retired
        return (s_o[c % 2], 16 * (c // 2 + 1))

    with nc.Block() as block:

        @block.gpsimd
        def _(g):
            # POOL starts behind the Bass-init memsets, so it gets the
            # non-latency-critical work: the W load and the result stores.
            g.dma_start(w32, wm2).then_inc(s_wld, 16)
            for c, (t, c0, w) in enumerate(CHUNKS):
                h = w // 2
                src = t_sb[c % 3].rearrange("r (g f) -> r g f", g=2)[:, :, 0:h]
                dst = od3[c, :, 0:w].rearrange("r (g f) -> r g f", g=2)
                g.wait_ge(s_dve, dve_i[("cpb", c)])
                g.dma_start(dst, src) \
                    ._wait_ge(s_act, act_i[("cpa", c)]) \
                    .then_inc(s_o[c % 2], 16)
            g.wait_ge(s_o[0], 16 * ((NC_CH + 1) // 2))
            g.wait_ge(s_o[1], 16 * (NC_CH // 2))

        @block.vector
        def _(v):
            v.tensor_copy(w16, w32)._wait_ge(s_wld, 16).then_inc(s_dve, 1)

            def emit_cpb(cc):
                _, _, w = CHUNKS[cc]
                if cc >= 3:
                    v.wait_ge(*o_wait(cc - 3))
                v.tensor_copy(t_sb[cc % 3][:, PRC:PRC + w // 2],
                              ps[cc % 3][:, PRC:PRC + w // 2]) \
                    ._wait_ge(s_pe, cc + 1).then_inc(s_dve, 1)

            def emit_pr(cc):
                _, _, w = CHUNKS[cc]
                if cc >= 3:
                    # pr slot free once mm(cc-3) retired; the same wait
                    # (s_pe >= cc-2) covers e1(cc-1)'s slot WAR
                    v.wait_ge(s_pe, cc - 2)
                v.tensor_mul(t_pr[cc % 3][:, 0:w],
                             t_in3[cc % N_SL][:, 1, 0:w],
                             t_in3[cc % N_SL][:, 2, 0:w]) \
                    ._wait_ge(*in_wait(cc)).then_inc(s_dve, 1)

            emit_pr(0)
            for c, (t, c0, w) in enumerate(CHUNKS):
                s3, s2 = c % 3, c % 2
                v.tensor_sub(t_d[s2][:, 0:w], t_lp[s2][:, 0:w],
                             t_lq[s2][:, 0:w]) \
                    ._wait_ge(s_act, act_i[("lp", c)]).then_inc(s_dve, 1)
                if c + 1 < NC_CH:
                    emit_pr(c + 1)
                else:
                    # no pr lookahead on the last chunk: cover e1's WAR
                    v.wait_ge(s_pe, c - 1)
                # same-engine RAW on d(c) needs an explicit retire wait
                v.wait_ge(s_dve, dve_i[("d", c)])
                v.tensor_mul(t_e1[s2][:, 0:w], t_pr[s3][:, 0:w],
                             t_d[s2][:, 0:w]).then_inc(s_dve, 1)
                if c >= 2:
                    emit_cpb(c - 2)
            emit_cpb(NC_CH - 2)
            emit_cpb(NC_CH - 1)

        @block.scalar
        def _(s):
            s.wait_ge(s_cst, 2)

            def emit_cpa(cc):
                _, _, w = CHUNKS[cc]
                if cc >= 3:
                    s.wait_ge(*o_wait(cc - 3))
                s.activation(t_sb[cc % 3][:, 0:w // 2],
                             ps[cc % 3][:, 0:w // 2], Copy) \
                    ._wait_ge(s_pe, cc + 1).then_inc(s_act, 1)

            def emit_lq(cc):
                _, _, w = CHUNKS[cc]
                s.activation(t_lq[cc % 2][:, 0:w],
                             t_in3[cc % N_SL][:, 0, 0:w], Ln, bias=EPS) \
                    ._wait_ge(*in_wait(cc)).then_inc(s_act, 1)

            emit_lq(0)
            for c, (t, c0, w) in enumerate(CHUNKS):
                s3, s2 = c % 3, c % 2
                s.activation(t_lp[s2][:, 0:w], t_pr[s3][:, 0:w], Ln,
                             bias=TINY) \
                    ._wait_ge(s_dve, dve_i[("pr", c)]).then_inc(s_act, 1)
                if c >= 2:
                    emit_cpa(c - 2)
                if c + 1 < NC_CH:
                    emit_lq(c + 1)
            emit_cpa(NC_CH - 2)
            emit_cpa(NC_CH - 1)

        @block.tensor
        def _(p):
            w8a = w16[:, 0:8]
            w8b = w16[:, 8:16]
            # ldweights of the first matmul precedes its inline wait; order
            # it after the w16 conversion explicitly
            p.wait_ge(s_dve, 1)
            for c, (t, c0, w) in enumerate(CHUNKS):
                s3, s2 = c % 3, c % 2
                sp3 = c % 3
                h = w // 2
                if c >= 3:
                    # psum slot free once cpa(c-3) retired (cpb via s_dve)
                    p.wait_ge(s_act, act_i[("cpa", c - 3)])
                p.matmul(ps[sp3][:, 0:h], w8a, t_e1[s2][:, 0:h],
                         start=True, stop=False) \
                    ._wait_ge(s_dve, dve_i[("e1", c)])
                p.matmul(ps[sp3][:, 0:h], w8b, t_e1[s2][:, h:w],
                         start=False, stop=True)
                p.matmul(ps[sp3][:, PRC:PRC + h], w8a, t_pr[s3][:, 0:h],
                         start=True, stop=False)
                p.matmul(ps[sp3][:, PRC:PRC + h], w8b, t_pr[s3][:, h:w],
                         start=False, stop=True).then_inc(s_pe, 1)

        @block.sync
        def _(sp):
            # SP's stream is otherwise empty, so its first instruction runs
            # right after the init barrier: issue the input loads here.
            for c, (t, c0, w) in enumerate(CHUNKS):
                if w == TILE_F:
                    # flat 2D AP: one contiguous 6KB row per partition
                    ins = sp.dma_start(t_in[c % N_SL], pk3[t, :, :])
                else:
                    ins = sp.dma_start(t_in3[c % N_SL][:, :, 0:w],
                                       pk4[t, :, :, c0:c0 + w])
                if c >= N_SL:
                    # input slot free once lp(c-N_SL) done (covers lq + pr)
                    ins._wait_ge(s_act, act_i[("lp", c - N_SL)])
                ins.then_inc(s_in[c % N_SL], 16)

    _check_one_wait(nc)
    return nc


def _get_program():
    if "nc" not in _CACHE:
        _CACHE["nc"] = _build_program()
    return _CACHE["nc"]


def _pack_inputs(yp, yt, w):
    """[N_TOTAL] f32 x3 -> per-core packed bf16 [t, p, {yp,yt,w}, f]."""
    import ml_dtypes

    def to_tiles(x):
        # element i_local = t*P*TILE_F + f*P + p  ->  [core, t, p, f]
        return x.reshape(N_CORES, N_TILES, TILE_F, P).transpose(0, 1, 3, 2)

    pk = np.stack([to_tiles(yp), to_tiles(yt), to_tiles(w)], axis=3)
    pk = np.ascontiguousarray(pk).astype(ml_dtypes.bfloat16)
    return pk.reshape(N_CORES, -1)


_WMAT = None


def _wmat():
    global _WMAT
    if _WMAT is None:
        wmat = np.zeros((P, 16), dtype=np.float32)
        for j in range(JBLK):
            wmat[BLK * j:BLK * (j + 1), j] = 1.0        # half 0 -> rows 0:4
            wmat[BLK * j:BLK * (j + 1), 12 + j] = 1.0   # half 1 -> rows 4:8
        _WMAT = wmat.reshape(-1)
    return _WMAT


def _run_device(yp, yt, w, trace=False):
    from concourse.bass_utils import run_bass_kernel_spmd

    nc = _get_program()
    pk = _pack_inputs(yp, yt, w)
    wmat = _wmat()
    in_maps = [{"pk": pk[k], "wm": wmat} for k in range(N_CORES)]
    res = run_bass_kernel_spmd(nc, in_maps, list(range(N_CORES)), trace=trace)

    bs1_parts, bs2_parts = [], []
    for r in res.results:
        dev = r["od"].reshape(NC_CH, 8, TILE_F)
        bs1 = np.empty(N_BLOCKS_LOCAL, dtype=np.float64)
        bs2 = np.empty(N_BLOCKS_LOCAL, dtype=np.float64)
        for c, (t, c0, w) in enumerate(CHUNKS):
            h = w // 2
            # psum row r = quad j + 4*half; block = t*4096 + (c0+h*half+f')*4+j
            base = t * (TILE_F * JBLK) + c0 * JBLK
            nblk = w * JBLK
            b1 = dev[c, :, 0:h].reshape(2, JBLK, h)
            b2 = dev[c, :, h:w].reshape(2, JBLK, h)
            bs1[base:base + nblk] = b1.transpose(0, 2, 1).reshape(-1)
            bs2[base:base + nblk] = b2.transpose(0, 2, 1).reshape(-1)
        bs1_parts.append(bs1)
        bs2_parts.append(bs2)
    return np.concatenate(bs1_parts), np.concatenate(bs2_parts), res


def kernel(y_pred, y_true, weight, segment_ptr, _trace=False):
    yp = np.ascontiguousarray(np.asarray(y_pred), dtype=np.float32).reshape(-1)
    yt = np.ascontiguousarray(np.asarray(y_true), dtype=np.float32).reshape(-1)
    w = np.ascontiguousarray(np.asarray(weight), dtype=np.float32).reshape(-1)
    ptr = np.asarray(segment_ptr).astype(np.int64).reshape(-1)
    n = yp.shape[0]
    G = ptr.shape[0] - 1
    assert n == N_TOTAL, f"kernel compiled for N={N_TOTAL}, got {n}"

    bs1, bs2, res = _run_device(yp, yt, w, trace=_trace)
    _CACHE["last_res"] = res

    # ---- host assembly in fp64 ----
    pre1 = np.empty(bs1.shape[0] + 1)
    pre1[0] = 0.0
    np.cumsum(bs1, dtype=np.float64, out=pre1[1:])
    pre2 = np.empty(bs2.shape[0] + 1)
    pre2[0] = 0.0
    np.cumsum(bs2, dtype=np.float64, out=pre2[1:])

    # clip ptr defensively to [0, n] (reference guarantees this range)
    ptrc = np.clip(ptr, 0, n)
    b_idx = ptrc // BLK
    r = ptrc - b_idx * BLK  # offset within block
    # fp64 partial sums over [ptr - r, ptr) for boundaries not block-aligned
    seg_off = np.concatenate([[0], np.cumsum(r)])
    tot = int(seg_off[-1])
    part1 = np.zeros(ptrc.shape[0])
    part2 = np.zeros(ptrc.shape[0])
    if tot > 0:
        idx = np.repeat(ptrc - r, r) + (np.arange(tot) - np.repeat(seg_off[:-1], r))
        pr_h = yt[idx].astype(np.float64) * w[idx].astype(np.float64)
        e1_h = pr_h * (np.log(pr_h + TINY) - np.log(yp[idx].astype(np.float64) + EPS))
        nz = r > 0
        red_idx = np.minimum(seg_off[:-1][nz], tot - 1).astype(np.int64)
        part1[nz] = np.add.reduceat(e1_h, red_idx)
        part2[nz] = np.add.reduceat(pr_h, red_idx)

    C1 = pre1[b_idx] + part1
    C2 = pre2[b_idx] + part2
    A = np.diff(C1)
    Bg = np.diff(C2)
    S = np.maximum(Bg, EPS)
    total = np.sum((A - Bg * np.log(S)) / S) / max(G, 1)
    return np.float32(total)


# revision 26
# speedup vs baseline: 1.2411x; 1.0564x over previous
"""Graphwise KL loss (segment_reduce) on 8 trn2 NeuronCores.

Strategy:
  Device (O(N) memory-bound work, data-parallel over 8 cores; each core
  streams a contiguous 1/8 slice, inputs host-packed to bf16):
    pr = y_true * weight                      (DVE bf16 2x)
    d  = ln(pr + 1e-37) - ln(y_pred + 1e-8)   (ACT Ln x2 -> bf16, DVE sub)
    e1 = pr * d                               (DVE bf16 2x)
    32-element block sums of e1 and pr        (PE matmul, block-diag ones)
  Host (O(num_graphs) metadata assembly, fp64): reconstruct per-segment
  sums A_g (e1) and B_g (pr) from device block sums + fp64 partial sums
  at segment boundaries; with S_g = max(B_g, EPS):
      total = mean_g (A_g - B_g * ln(S_g)) / S_g

  Inputs are packed into ONE interleaved bf16 DRAM tensor per core laid
  out [tile, partition, {yp,yt,w}, col] with element i = 128*col + part
  inside a tile, so each 32-element block sits in one partition quad and
  PE reduces blocks via matmul.  The stationary is a [128,16] pair of
  block-diagonal ones matrices (cols 0:8 -> psum rows 0:4 for the first
  half of a chunk's columns, cols 8:16 -> rows 4:8 for the second half)
  so a whole chunk accumulates into one [8, w] PSUM region: e1 sums in
  cols 0:w/2, pr sums in w/2:w.  PSUM is evacuated split: ACT copies the
  e1 half, DVE the pr half, both skewed one chunk late so they never
  block the next chunk's front end.  One load DMA + one store DMA per
  chunk; 3-deep input buffers keep the 16 SDMA engines streaming.

  The work is cut into chunks: 7 full tiles of 1024 columns plus 4
  quarter tiles of 256 at the end, so the serial drain chain after the
  last load is short.

  Raw Bass (no Tile): every op carries at most ONE inline sync wait
  (walrus cap); extra deps use standalone wait_ge instructions.
    POOL: load DMAs    DVE: pr/d/e1 + pr-psum evac
    ACT:  Ln x2 + e1-psum evac    PE: 4 matmuls/chunk    SP: store DMAs
"""

import numpy as np

N_TOTAL = 8388608
N_CORES = 8
N_LOCAL = N_TOTAL // N_CORES      # 1048576
P = 128
TILE_F = 1024                     # columns per full tile
N_TILES = N_LOCAL // (P * TILE_F)  # 8
BLK = 32
JBLK = P // BLK                   # 4 blocks per column
N_BLOCKS_LOCAL = N_LOCAL // BLK   # 32768
EPS = 1e-8
TINY = 1e-37

# chunk list: (tile, col0, width) — last tile split into quarters
CHUNKS = [(0, 0, TILE_F // 2), (0, TILE_F // 2, TILE_F // 2)]
CHUNKS += [(t, 0, TILE_F) for t in range(1, N_TILES - 1)]
CHUNKS += [(N_TILES - 1, c0, TILE_F // 2)
           for c0 in range(0, TILE_F, TILE_F // 2)]
NC_CH = len(CHUNKS)               # 12
PRC = TILE_F // 2                 # psum col of the pr group (bank-aligned)

_CACHE = {}


def _check_one_wait(nc):
    """Assert no non-EventSemaphore instruction carries more than one wait."""
    bad = []
    for f in nc.m.functions:
        for bb in f.blocks:
            for inst in bb.instructions:
                si = inst.sync_info
                if si and si.on_wait and len(si.on_wait) > 1:
                    if "EventSem" not in type(inst).__name__:
                        bad.append((type(inst).__name__, inst.name, len(si.on_wait)))
    assert not bad, f"multi-wait instructions remain: {bad}"


def _build_program():
    import concourse.bass as bass
    import concourse.mybir as mybir

    f32 = mybir.dt.float32
    bf16 = mybir.dt.bfloat16
    Ln = mybir.ActivationFunctionType.Ln
    Copy = mybir.ActivationFunctionType.Copy

    nc = bass.Bass()

    # Const APs for the Ln biases (activation() looks these up by value).
    # Emitted on the otherwise-idle DVE so POOL can start load DMAs at once.
    s_cst = nc.alloc_semaphore("s_cst")
    for val in (TINY, EPS):
        ct = nc.alloc_sbuf_tensor(f"const-f32-{val}", [128, 1], f32)
        nc.vector.memset(ct.ap(), val).then_inc(s_cst, 1)
        nc.const_aps.aps[(f32, val)] = ct.ap()

    pk = nc.declare_dram_parameter("pk", [3 * N_LOCAL], bf16, isOutput=False)
    wm = nc.declare_dram_parameter("wm", [P * 16], f32, isOutput=False)
    od = nc.declare_dram_parameter("od", [NC_CH * 8 * TILE_F], f32,
                                   isOutput=True)

    pk3 = pk[:].rearrange("(t p cf) -> t p cf", p=P, cf=3 * TILE_F)
    pk4 = pk[:].rearrange("(t p c f) -> t p c f", p=P, c=3, f=TILE_F)
    wm2 = wm[:].rearrange("(p j) -> p j", j=16)
    od3 = od[:].rearrange("(i r f) -> i r f", r=8, f=TILE_F)

    N_SL = 4
    t_in = [nc.alloc_sbuf_tensor(f"t_in{i}", [P, 3 * TILE_F], bf16).ap()
            for i in range(N_SL)]
    t_in3 = [a.rearrange("p (c f) -> p c f", c=3) for a in t_in]
    t_pr = [nc.alloc_sbuf_tensor(f"t_pr{i}", [P, TILE_F], bf16).ap()
            for i in range(3)]
    t_lp = [nc.alloc_sbuf_tensor(f"t_lp{i}", [P, TILE_F], bf16).ap()
            for i in range(2)]
    t_lq = [nc.alloc_sbuf_tensor(f"t_lq{i}", [P, TILE_F], bf16).ap()
            for i in range(2)]
    t_d = [nc.alloc_sbuf_tensor(f"t_d{i}", [P, TILE_F], bf16).ap()
           for i in range(2)]
    t_e1 = [nc.alloc_sbuf_tensor(f"t_e1{i}", [P, TILE_F], bf16).ap()
            for i in range(2)]
    t_sb = [nc.alloc_sbuf_tensor(f"t_sb{i}", [8, TILE_F], f32).ap()
            for i in range(3)]
    w32 = nc.alloc_sbuf_tensor("w32", [P, 16], f32).ap()
    w16 = nc.alloc_sbuf_tensor("w16", [P, 16], bf16).ap()

    ps = [nc.alloc_psum_tensor(f"ps{i}", [8, TILE_F], f32).ap()
          for i in range(3)]

    s_in = [nc.alloc_semaphore(f"s_in{i}") for i in range(N_SL)]  # +16/load
    s_o = [nc.alloc_semaphore(f"s_o{i}") for i in range(2)]     # +16 per store
    s_wld = nc.alloc_semaphore("s_wld")                         # +16 W load
    s_dve = nc.alloc_semaphore("s_dve")  # +1 per DVE op
    s_act = nc.alloc_semaphore("s_act")  # +1 per ACT op
    s_pe = nc.alloc_semaphore("s_pe")    # +1 per chunk (4th matmul)

    # Op indices along each engine's in-order stream.
    # DVE order hoists pr(c+1) right after d(c) so ACT's lp(c+1) can start
    # while DVE still runs e1(c)/cpb(c-1) — this breaks the cross-engine
    # latency loop that otherwise sets the cycle time.
    dve_order = [("pr", 0)]
    for c in range(NC_CH):
        dve_order.append(("d", c))
        if c + 1 < NC_CH:
            dve_order.append(("pr", c + 1))
        dve_order.append(("e1", c))
        if c >= 2:
            dve_order.append(("cpb", c - 2))
    dve_order.append(("cpb", NC_CH - 2))
    dve_order.append(("cpb", NC_CH - 1))
    dve_i = {}
    n = 1                                 # w16 convert = 1
    for o in dve_order:
        n += 1
        dve_i[o] = n

    # ACT order: lq runs one chunk ahead so lp's wait on pr(c) never blocks
    # the next chunk's front end; cpa trails one chunk behind.
    act_order = [("lq", 0)]
    for c in range(NC_CH):
        act_order.append(("lp", c))
        if c >= 2:
            act_order.append(("cpa", c - 2))
        if c + 1 < NC_CH:
            act_order.append(("lq", c + 1))
    act_order.append(("cpa", NC_CH - 2))
    act_order.append(("cpa", NC_CH - 1))
    act_i = {}
    n = 0
    for o in act_order:
        n += 1
        act_i[o] = n

    def in_wait(c):
        return (s_in[c % N_SL], 16 * (c // N_SL + 1))

    def o_wait(c):
        # store of chunk c retired
        return (s_o[c % 2], 16 * (c // 2 + 1))

    with nc.Block(no_gpsimd_drain=True) as block:

        @block.gpsimd
        def _(g):
            # POOL starts behind the Bass-init memsets, so it gets the
            # non-latency-critical work: the W load and the result stores.
            g.dma_start(w32, wm2).then_inc(s_wld, 16)
            for c, (t, c0, w) in enumerate(CHUNKS):
                h = w // 2
                src = t_sb[c % 3].rearrange("r (g f) -> r g f", g=2)[:, :, 0:h]
                dst = od3[c, :, 0:w].rearrange("r (g f) -> r g f", g=2)
                g.wait_ge(s_dve, dve_i[("cpb", c)])
                g.dma_start(dst, src) \
                    ._wait_ge(s_act, act_i[("cpa", c)]) \
                    .then_inc(s_o[c % 2], 16)
            g.wait_ge(s_o[0], 16 * ((NC_CH + 1) // 2))
            g.wait_ge(s_o[1], 16 * (NC_CH // 2))

        @block.vector
        def _(v):
            v.tensor_copy(w16, w32)._wait_ge(s_wld, 16).then_inc(s_dve, 1)

            def emit_cpb(cc):
                _, _, w = CHUNKS[cc]
                if cc >= 3:
                    v.wait_ge(*o_wait(cc - 3))
                v.tensor_copy(t_sb[cc % 3][:, PRC:PRC + w // 2],
                              ps[cc % 3][:, PRC:PRC + w // 2]) \
                    ._wait_ge(s_pe, cc + 1).then_inc(s_dve, 1)

            def emit_pr(cc):
                _, _, w = CHUNKS[cc]
                if cc >= 3:
                    # pr slot free once mm(cc-3) retired; the same wait
                    # (s_pe >= cc-2) covers e1(cc-1)'s slot WAR
                    v.wait_ge(s_pe, cc - 2)
                v.tensor_mul(t_pr[cc % 3][:, 0:w],
                             t_in3[cc % N_SL][:, 1, 0:w],
                             t_in3[cc % N_SL][:, 2, 0:w]) \
                    ._wait_ge(*in_wait(cc)).then_inc(s_dve, 1)

            emit_pr(0)
            for c, (t, c0, w) in enumerate(CHUNKS):
                s3, s2 = c % 3, c % 2
                v.tensor_sub(t_d[s2][:, 0:w], t_lp[s2][:, 0:w],
                             t_lq[s2][:, 0:w]) \
                    ._wait_ge(s_act, act_i[("lp", c)]).then_inc(s_dve, 1)
                if c + 1 < NC_CH:
                    emit_pr(c + 1)
                else:
                    # no pr lookahead on the last chunk: cover e1's WAR
                    v.wait_ge(s_pe, c - 1)
                # same-engine RAW on d(c) needs an explicit retire wait
                v.wait_ge(s_dve, dve_i[("d", c)])
                v.tensor_mul(t_e1[s2][:, 0:w], t_pr[s3][:, 0:w],
                             t_d[s2][:, 0:w]).then_inc(s_dve, 1)
                if c >= 2:
                    emit_cpb(c - 2)
            emit_cpb(NC_CH - 2)
            emit_cpb(NC_CH - 1)

        @block.scalar
        def _(s):
            s.wait_ge(s_cst, 2)

            def emit_cpa(cc):
                _, _, w = CHUNKS[cc]
                if cc >= 3:
                    s.wait_ge(*o_wait(cc - 3))
                s.activation(t_sb[cc % 3][:, 0:w // 2],
                             ps[cc % 3][:, 0:w // 2], Copy) \
                    ._wait_ge(s_pe, cc + 1).then_inc(s_act, 1)

            def emit_lq(cc):
                _, _, w = CHUNKS[cc]
                s.activation(t_lq[cc % 2][:, 0:w],
                             t_in3[cc % N_SL][:, 0, 0:w], Ln, bias=EPS) \
                    ._wait_ge(*in_wait(cc)).then_inc(s_act, 1)

            emit_lq(0)
            for c, (t, c0, w) in enumerate(CHUNKS):
                s3, s2 = c % 3, c % 2
                s.activation(t_lp[s2][:, 0:w], t_pr[s3][:, 0:w], Ln,
                             bias=TINY) \
                    ._wait_ge(s_dve, dve_i[("pr", c)]).then_inc(s_act, 1)
                if c >= 2:
                    emit_cpa(c - 2)
                if c + 1 < NC_CH:
                    emit_lq(c + 1)
            emit_cpa(NC_CH - 2)
            emit_cpa(NC_CH - 1)

        @block.tensor
        def _(p):
            w8a = w16[:, 0:8]
            w8b = w16[:, 8:16]
            # ldweights of the first matmul precedes its inline wait; order
            # it after the w16 conversion explicitly
            p.wait_ge(s_dve, 1)
            for c, (t, c0, w) in enumerate(CHUNKS):
                s3, s2 = c % 3, c % 2
                sp3 = c % 3
                h = w // 2
                if c >= 3:
                    # psum slot free once cpa(c-3) retired (cpb via s_dve)
                    p.wait_ge(s_act, act_i[("cpa", c - 3)])
                p.matmul(ps[sp3][:, 0:h], w8a, t_e1[s2][:, 0:h],
                         start=True, stop=False) \
                    ._wait_ge(s_dve, dve_i[("e1", c)])
                p.matmul(ps[sp3][:, 0:h], w8b, t_e1[s2][:, h:w],
                         start=False, stop=True)
                p.matmul(ps[sp3][:, PRC:PRC + h], w8a, t_pr[s3][:, 0:h],
                         start=True, stop=False)
                p.matmul(ps[sp3][:, PRC:PRC + h], w8b, t_pr[s3][:, h:w],
                         start=False, stop=True).then_inc(s_pe, 1)

        @block.sync
        def _(sp):
            # SP's stream is otherwise empty, so its first instruction runs
            # right after the init barrier: issue the input loads here.
            for c, (t, c0, w) in enumerate(CHUNKS):
                if w == TILE_F:
                    # flat 2D AP: one contiguous 6KB row per partition
                    ins = sp.dma_start(t_in[c % N_SL], pk3[t, :, :])
                else:
                    ins = sp.dma_start(t_in3[c % N_SL][:, :, 0:w],
                                       pk4[t, :, :, c0:c0 + w])
                if c >= N_SL:
                    # input slot free once lp(c-N_SL) done (covers lq + pr)
                    ins._wait_ge(s_act, act_i[("lp", c - N_SL)])
                ins.then_inc(s_in[c % N_SL], 16)

    _check_one_wait(nc)
    return nc


def _get_program():
    if "nc" not in _CACHE:
        _CACHE["nc"] = _build_program()
    return _CACHE["nc"]


def _pack_inputs(yp, yt, w):
    """[N_TOTAL] f32 x3 -> per-core packed bf16 [t, p, {yp,yt,w}, f]."""
    import ml_dtypes

    def to_tiles(x):
        # element i_local = t*P*TILE_F + f*P + p  ->  [core, t, p, f]
        return x.reshape(N_CORES, N_TILES, TILE_F, P).transpose(0, 1, 3, 2)

    pk = np.stack([to_tiles(yp), to_tiles(yt), to_tiles(w)], axis=3)
    pk = np.ascontiguousarray(pk).astype(ml_dtypes.bfloat16)
    return pk.reshape(N_CORES, -1)


_WMAT = None


def _wmat():
    global _WMAT
    if _WMAT is None:
        wmat = np.zeros((P, 16), dtype=np.float32)
        for j in range(JBLK):
            wmat[BLK * j:BLK * (j + 1), j] = 1.0        # half 0 -> rows 0:4
            wmat[BLK * j:BLK * (j + 1), 12 + j] = 1.0   # half 1 -> rows 4:8
        _WMAT = wmat.reshape(-1)
    return _WMAT


def _run_device(yp, yt, w, trace=False):
    from concourse.bass_utils import run_bass_kernel_spmd

    nc = _get_program()
    pk = _pack_inputs(yp, yt, w)
    wmat = _wmat()
    in_maps = [{"pk": pk[k], "wm": wmat} for k in range(N_CORES)]
    res = run_bass_kernel_spmd(nc, in_maps, list(range(N_CORES)), trace=trace)

    bs1_parts, bs2_parts = [], []
    for r in res.results:
        dev = r["od"].reshape(NC_CH, 8, TILE_F)
        bs1 = np.empty(N_BLOCKS_LOCAL, dtype=np.float64)
        bs2 = np.empty(N_BLOCKS_LOCAL, dtype=np.float64)
        for c, (t, c0, w) in enumerate(CHUNKS):
            h = w // 2
            # psum row r = quad j + 4*half; block = t*4096 + (c0+h*half+f')*4+j
            base = t * (TILE_F * JBLK) + c0 * JBLK
            nblk = w * JBLK
            b1 = dev[c, :, 0:h].reshape(2, JBLK, h)
            b2 = dev[c, :, h:w].reshape(2, JBLK, h)
            bs1[base:base + nblk] = b1.transpose(0, 2, 1).reshape(-1)
            bs2[base:base + nblk] = b2.transpose(0, 2, 1).reshape(-1)
        bs1_parts.append(bs1)
        bs2_parts.append(bs2)
    return np.concatenate(bs1_parts), np.concatenate(bs2_parts), res


def kernel(y_pred, y_true, weight, segment_ptr, _trace=False):
    yp = np.ascontiguousarray(np.asarray(y_pred), dtype=np.float32).reshape(-1)
    yt = np.ascontiguousarray(np.asarray(y_true), dtype=np.float32).reshape(-1)
    w = np.ascontiguousarray(np.asarray(weight), dtype=np.float32).reshape(-1)
    ptr = np.asarray(segment_ptr).astype(np.int64).reshape(-1)
    n = yp.shape[0]
    G = ptr.shape[0] - 1
    assert n == N_TOTAL, f"kernel compiled for N={N_TOTAL}, got {n}"

    bs1, bs2, res = _run_device(yp, yt, w, trace=_trace)
    _CACHE["last_res"] = res

    # ---- host assembly in fp64 ----
    pre1 = np.empty(bs1.shape[0] + 1)
    pre1[0] = 0.0
    np.cumsum(bs1, dtype=np.float64, out=pre1[1:])
    pre2 = np.empty(bs2.shape[0] + 1)
    pre2[0] = 0.0
    np.cumsum(bs2, dtype=np.float64, out=pre2[1:])

    # clip ptr defensively to [0, n] (reference guarantees this range)
    ptrc = np.clip(ptr, 0, n)
    b_idx = ptrc // BLK
    r = ptrc - b_idx * BLK  # offset within block
    # fp64 partial sums over [ptr - r, ptr) for boundaries not block-aligned
    seg_off = np.concatenate([[0], np.cumsum(r)])
    tot = int(seg_off[-1])
    part1 = np.zeros(ptrc.shape[0])
    part2 = np.zeros(ptrc.shape[0])
    if tot > 0:
        idx = np.repeat(ptrc - r, r) + (np.arange(tot) - np.repeat(seg_off[:-1], r))
        pr_h = yt[idx].astype(np.float64) * w[idx].astype(np.float64)
        e1_h = pr_h * (np.log(pr_h + TINY) - np.log(yp[idx].astype(np.float64) + EPS))
        nz = r > 0
        red_idx = np.minimum(seg_off[:-1][nz], tot - 1).astype(np.int64)
        part1[nz] = np.add.reduceat(e1_h, red_idx)
        part2[nz] = np.add.reduceat(pr_h, red_idx)

    C1 = pre1[b_idx] + part1
    C2 = pre2[b_idx] + part2
    A = np.diff(C1)
    Bg = np.diff(C2)
    S = np.maximum(Bg, EPS)
    total = np.sum((A - Bg * np.log(S)) / S) / max(G, 1)
    return np.float32(total)
